# revision 22
# baseline (speedup 1.0000x reference)
"""Trainium2 Bass kernel for DeepUnfoldingNet CTG local-window attention.

Math (per view v, per pixel p):
  theta = Wt @ A ;  phi = Wp @ x1 ;  g = Wg @ x1   (1x1 convs, C=48)
  S[p, q] = theta(p) . phi(q)  for q in the 9x9 window around p
  att = softmax_q(S);  out = Ww @ (sum_q att * g(q)) + A

Folded on HOST (the convs are tiny 48x48 GEMMs):
  tw = (Wt^T Wp)^T A        -> S = tw(p) . x1(q)
  gt = ((Ww Wg) x1)^T + ones row (softmax denominator), q-major per seg.

Sharding: H=128 -> 8 strips of 16 rows (one per core), all 9 views per core;
warped input gets a 4-pixel zero halo (matches torch-unfold zero padding).

Device tiling per view: 16 P-tiles of 8x16 pixels (=128 partitions). Each
P-tile attends over a 16x24 padded Q-window = 3 q-chunks of 128 (16 rows x 8
cols). Scores are computed transposed (S_T[q, p]) into one PSUM bank per
tile as [128q, 3*128]:
  S_T chunk k = x1[48, seg]^T-mm-tw[48, ptile]  (seg = 17*tr + 2*tc + k)
  += mask (-1e9 outside window) in ONE GpSimd tensor_tensor [128, 384]
  E = exp in ONE ScalarE activation [128, 384] (PSUM -> SBUF bf16)
  out[p, 0:49] += E_k^T-mm-gt[seg] (3-chunk PSUM accumulation, packed
  49-col regions per tile-row bank)
PE emission is software-pipelined (S of tile t+2 ahead of agg of tile t) so
the tensor engine never waits on the exp chain. Host does padding, weight
folding, layout chunking, final divide, transpose, residual.
"""

import numpy as np
import ml_dtypes

_BF16 = ml_dtypes.bfloat16

_N, _C, _H, _W = 9, 48, 128, 128
_NCORES = 8
_SR = 16            # strip rows per core
_NPIX = _SR * _W    # 2048 pixels per strip
_NSEG = 34          # 2 tile-rows x 17 col-bands of 16x8 q-chunks
_CA = _C + 1        # 48 channels + ones (denominator)

_nc_cache = []
_last_res = None


def _build_nc():
    import concourse.bacc as bacc
    import concourse.mybir as mybir
    from concourse import tile
    from contextlib import ExitStack

    f32 = mybir.dt.float32
    bf16 = mybir.dt.bfloat16
    AF = mybir.ActivationFunctionType
    ALU = mybir.AluOpType

    nc = bacc.Bacc()
    # tw: tile-major pixels (tr, tcol, pr, pc); x1: chunk-major q (seg, q)
    tw_d = nc.dram_tensor("tw", [_N, _C, _NPIX], bf16, kind="ExternalInput")
    x1_d = nc.dram_tensor("x1", [_N, _C, _NSEG, 128], bf16,
                          kind="ExternalInput")
    gt_d = nc.dram_tensor("gt", [_N, 128, _NSEG, _CA], bf16,
                          kind="ExternalInput")
    msk_d = nc.dram_tensor("msk", [128, 768], bf16, kind="ExternalInput")
    # out[v, tile-row, p(128), tile(8)*49+c]: agg accumulators + denominator
    out_d = nc.dram_tensor("out", [_N, 2, 128, 8 * _CA], bf16,
                           kind="ExternalOutput")

    with tile.TileContext(nc) as tc, ExitStack() as ctx:
        const = ctx.enter_context(tc.tile_pool(name="const", bufs=1))
        vin = ctx.enter_context(tc.tile_pool(name="vin", bufs=3))
        esb = ctx.enter_context(tc.tile_pool(name="esb", bufs=3))
        osb = ctx.enter_context(tc.tile_pool(name="osb", bufs=2))
        ps_s = ctx.enter_context(tc.tile_pool(name="ps_s", bufs=3,
                                              space="PSUM"))
        ps_o = ctx.enter_context(tc.tile_pool(name="ps_o", bufs=1,
                                              space="PSUM"))

        msk = const.tile([128, 768], bf16)
        nc.sync.dma_start(msk[:], msk_d[:])
        # prime DVE's vector clock on the mask DMA: the HW TensorTensor
        # instruction has a single sync-wait slot, so the first mask-mult must
        # not need both a DMA wait and an ACT wait.
        dummy = const.tile([128, 1], bf16)
        nc.vector.tensor_copy(dummy[:], msk[:, 0:1])

        vins = [None] * _N
        pso = [None, None]
        epair = [None] * 16

        def fetch(v):
            # tw/x1 (needed first, hardware DGE via Sync, split per
            # tile-row); gt is only read by the later agg matmuls and out
            # goes the other way, both on GpSimd's software DGE so a
            # waiting descriptor never blocks the Sync queue's prefetches.
            tw = vin.tile([_C, _NPIX], bf16, tag="tw", name="tw")
            x1 = vin.tile([_C, _NSEG, 128], bf16, tag="x1", name="x1")
            gt = vin.tile([128, _NSEG, _CA], bf16, tag="gt", name="gt")
            for tr in range(2):
                nc.sync.dma_start(tw[:, 1024 * tr:1024 * (tr + 1)],
                                  tw_d[v, :, 1024 * tr:1024 * (tr + 1)])
                nc.sync.dma_start(x1[:, 17 * tr:17 * (tr + 1), :],
                                  x1_d[v, :, 17 * tr:17 * (tr + 1), :])
            nc.gpsimd.dma_start(gt[:], gt_d[v])
            vins[v] = (tw, x1, gt)

        def s_pair(gp):
            # scores for tiles 2j, 2j+1 of view gp//8 into one 2-bank PSUM
            # tile [128q, 2*384] (tile slot stride 384); the pair shares its
            # boundary seg, so chunks (2j,k=2) and (2j+1,k=0) merge into
            # ONE matmul with a 256-wide moving operand. Then one big
            # exp + one big mask-multiply for the pair.
            v, j = gp // 8, gp % 8
            tw, x1, _ = vins[v]
            sc = ps_s.tile([128, 768], f32, tag="scat", name="sc")
            t0 = 2 * j
            tr, tc_ = t0 // 8, t0 % 8
            sb = 17 * tr + 2 * tc_          # seg of (t0, k=0)
            nc.tensor.matmul(sc[:, 0:128], lhsT=x1[:, sb, :],
                             rhs=tw[:, 128 * t0:128 * t0 + 128],
                             start=True, stop=True)
            nc.tensor.matmul(sc[:, 128:256], lhsT=x1[:, sb + 1, :],
                             rhs=tw[:, 128 * t0:128 * t0 + 128],
                             start=True, stop=True)
            nc.tensor.matmul(sc[:, 256:512], lhsT=x1[:, sb + 2, :],
                             rhs=tw[:, 128 * t0:128 * t0 + 256],
                             start=True, stop=True)
            nc.tensor.matmul(sc[:, 512:640], lhsT=x1[:, sb + 3, :],
                             rhs=tw[:, 128 * t0 + 128:128 * t0 + 256],
                             start=True, stop=True)
            nc.tensor.matmul(sc[:, 640:768], lhsT=x1[:, sb + 4, :],
                             rhs=tw[:, 128 * t0 + 128:128 * t0 + 256],
                             start=True, stop=True)
            # exp (PSUM -> SBUF bf16), then 0/1 window mask multiply
            # on DVE: exp(S)*0 == exp(S - 1e9) for out-of-window q
            e = esb.tile([128, 768], bf16, tag="e", name="e")
            epair[gp % 16] = e
            nc.scalar.activation(e[:], sc[:], AF.Exp)
            nc.vector.tensor_tensor(out=e[:], in0=e[:], in1=msk[:],
                                    op=ALU.mult)

        def a_phase(gp, slot):
            v, t = gp // 8, 2 * (gp % 8) + slot
            _, _, gt = vins[v]
            tr, tc_ = t // 8, t % 8
            if tc_ == 0:
                pso[tr] = ps_o.tile([128, 8 * _CA], f32,
                                    tag=f"pso{tr}", name=f"pso{tr}")
            po = pso[tr]
            e = epair[gp % 16]
            for k in range(3):
                seg = 17 * tr + 2 * tc_ + k
                nc.tensor.matmul(
                    po[:, _CA * tc_:_CA * (tc_ + 1)],
                    lhsT=e[:, 384 * slot + 128 * k:
                           384 * slot + 128 * (k + 1)],
                    rhs=gt[:, seg, :],
                    start=(k == 0), stop=(k == 2))
            if tc_ == 7:
                # drain the finished tile-row bank to SBUF + DRAM
                o = osb.tile([128, 8 * _CA], bf16, tag=f"ob{tr}",
                             name=f"ob{tr}")
                nc.vector.tensor_copy(o[:], po[:])
                nc.gpsimd.dma_start(out_d[v, tr], o[:])

        # one global software pipeline over all 9*8 pairs: the PE stays 2
        # pairs ahead of the exp chain with no drain at view boundaries
        fetch(0)
        fetch(1)
        for gp in range(74):
            if gp < 72:
                if gp % 8 == 0 and gp // 8 + 2 < _N:
                    fetch(gp // 8 + 2)
                s_pair(gp)
            if gp >= 2:
                a_phase(gp - 2, 0)
                a_phase(gp - 2, 1)
    if not nc.is_finalized():
        nc.finalize()
    return nc


def _masks() -> np.ndarray:
    """mask[q=qr*8+qc, chunk*128 + p=pr*16+pc]: 1 if q in p's 9x9 window."""
    qr = (np.arange(128) // 8)[:, None]
    qc = (np.arange(128) % 8)[:, None]
    pr = (np.arange(128) // 16)[None, :]
    pc = (np.arange(128) % 16)[None, :]
    m = np.zeros((128, 3, 128), np.float32)
    for kk in range(3):
        valid = ((qr - pr >= 0) & (qr - pr <= 8)
                 & (qc + 8 * kk - pc >= 0) & (qc + 8 * kk - pc <= 8))
        m[:, kk, :][valid] = 1.0
    m = m.reshape(128, 384)
    return np.concatenate([m, m], axis=1).astype(_BF16)  # [128, 768]


def _seg_chunk(img: np.ndarray, r0: int) -> np.ndarray:
    """img [n, ch, 136, 136] padded -> [n, ch, 34, 128] seg-major chunks
    for the 16-row strip starting at unpadded row r0."""
    n, ch = img.shape[:2]
    xs = img[:, :, r0:r0 + _SR + 8, :]                 # [n,ch,24,136]
    segs = np.empty((n, ch, _NSEG, 128), np.float32)
    for tr in range(2):
        sl = xs[:, :, 8 * tr:8 * tr + 16, :]           # [n,ch,16,136]
        sl = sl.reshape(n, ch, 16, 17, 8).transpose(0, 1, 3, 2, 4)
        segs[:, :, 17 * tr:17 * (tr + 1), :] = sl.reshape(n, ch, 17, 128)
    return segs


def kernel(**inputs) -> np.ndarray:
    A = np.asarray(inputs["A"], np.float32)            # [1,9,48,128,128]
    wc = np.asarray(inputs["warped_c"], np.float32)    # [1,9,48,128,128]
    Wt = np.asarray(inputs["Wt"], np.float32)
    Wp = np.asarray(inputs["Wp"], np.float32)
    Wg = np.asarray(inputs["Wg"], np.float32)
    Ww = np.asarray(inputs["Ww"], np.float32)

    Wtp = Wt.T @ Wp                                    # S = tw^T x1
    Wwg = Ww @ Wg
    wwgt = np.zeros((_CA, _CA), np.float32)
    wwgt[:_C, :_C] = Wwg.T
    wwgt[_C, _C] = 1.0

    # padded warped input + ones channel: [9, 49, 136, 136]
    x1p = np.pad(wc[0], ((0, 0), (0, 0), (4, 4), (4, 4)))
    x1aug = np.concatenate(
        [x1p, np.ones((_N, 1, _H + 8, _W + 8), np.float32)], axis=1)
    # g image (q-side values + ones) on host: tiny 49x49 GEMM per pixel
    gimg = np.einsum('cj,vchw->vjhw', wwgt, x1aug, optimize=True)

    # theta-folded query image on host: [9, 48, 128, 128]
    twimg = np.einsum('co,vchw->vohw', Wtp, A[0], optimize=True)

    msk = _masks()
    in_maps = []
    for cid in range(_NCORES):
        r0 = cid * _SR
        # tw tile-major: (tr, tc, pr, pc) -> [9,48,2048]
        strip = twimg[:, :, r0:r0 + _SR, :]            # [9,48,16,128]
        tw = strip.reshape(_N, _C, 2, 8, 8, 16).transpose(0, 1, 2, 4, 3, 5)
        tw = np.ascontiguousarray(tw.reshape(_N, _C, _NPIX)).astype(_BF16)
        x1segs = _seg_chunk(x1p, r0)                   # [9,48,34,128]
        gtsegs = _seg_chunk(gimg, r0)                  # [9,49,34,128]
        gt = np.ascontiguousarray(
            gtsegs.transpose(0, 3, 2, 1)).astype(_BF16)  # [9,128,34,49]
        in_maps.append({
            "tw": tw,
            "x1": np.ascontiguousarray(x1segs).astype(_BF16),
            "gt": gt,
            "msk": msk,
        })

    from concourse.bass_utils import run_bass_kernel_spmd
    if not _nc_cache:
        _nc_cache.append(_build_nc())
    res = run_bass_kernel_spmd(_nc_cache[0], in_maps, list(range(_NCORES)))
    global _last_res
    _last_res = res

    strips = []
    for cid in range(_NCORES):
        o = np.asarray(res.results[cid]["out"], np.float32)
        # o[v, tr, p=pr*16+pc, tc*49 + c]
        o = o.reshape(_N, 2, 8, 16, 8, _CA)            # v, tr, pr, pc, tc, c
        att = o[..., :_C] / o[..., _C:]
        # -> [v, c, tr, pr, tc, pc] -> [v, c, 16, 128]
        att = att.transpose(0, 5, 1, 2, 4, 3).reshape(_N, _C, _SR, _W)
        strips.append(att)
    att_full = np.concatenate(strips, axis=2)[None]    # [1,9,48,128,128]
    return (A + att_full).astype(np.float32)


# revision 26
# speedup vs baseline: 1.2086x; 1.2086x over previous
"""Trainium2 Bass kernel for DeepUnfoldingNet CTG local-window attention.

Math (per view v, per pixel p):
  theta = Wt @ A ;  phi = Wp @ x1 ;  g = Wg @ x1   (1x1 convs, C=48)
  S[p, q] = theta(p) . phi(q)  for q in the 9x9 window around p
  att = softmax_q(S);  out = Ww @ (sum_q att * g(q)) + A

Folded on HOST (the convs are tiny 48x48 GEMMs):
  tw = (Wt^T Wp)^T A        -> S = tw(p) . x1(q)
  gt = ((Ww Wg) x1)^T + ones row (softmax denominator), q-major per seg.

Sharding: H=128 -> 8 strips of 16 rows (one per core), all 9 views per core;
warped input gets a 4-pixel zero halo (matches torch-unfold zero padding).

Device tiling per view: 16 P-tiles of 8x16 pixels (=128 partitions). Each
P-tile attends over a 16x24 padded Q-window = 3 q-chunks of 128 (16 rows x 8
cols). Scores are computed transposed (S_T[q, p]) into one PSUM bank per
tile as [128q, 3*128]:
  S_T chunk k = x1[48, seg]^T-mm-tw[48, ptile]  (seg = 17*tr + 2*tc + k)
  += mask (-1e9 outside window) in ONE GpSimd tensor_tensor [128, 384]
  E = exp in ONE ScalarE activation [128, 384] (PSUM -> SBUF bf16)
  out[p, 0:49] += E_k^T-mm-gt[seg] (3-chunk PSUM accumulation, packed
  49-col regions per tile-row bank)
PE emission is software-pipelined (S of tile t+2 ahead of agg of tile t) so
the tensor engine never waits on the exp chain. Host does padding, weight
folding, layout chunking, final divide, transpose, residual.
"""

import numpy as np
import ml_dtypes

_BF16 = ml_dtypes.bfloat16

_N, _C, _H, _W = 9, 48, 128, 128
_NCORES = 8
_SR = 16            # strip rows per core
_NPIX = _SR * _W    # 2048 pixels per strip
_NSEG = 34          # 2 tile-rows x 17 col-bands of 16x8 q-chunks
_CA = _C + 1        # 48 channels + ones (denominator)

_nc_cache = []
_last_res = None


def _build_nc():
    import concourse.bacc as bacc
    import concourse.mybir as mybir
    from concourse import tile
    from contextlib import ExitStack

    f32 = mybir.dt.float32
    bf16 = mybir.dt.bfloat16
    AF = mybir.ActivationFunctionType
    ALU = mybir.AluOpType

    nc = bacc.Bacc()
    # tw: tile-major pixels (tr, tcol, pr, pc); x1: chunk-major q (seg, q)
    tw_d = nc.dram_tensor("tw", [_N, _C, _NPIX], bf16, kind="ExternalInput")
    x1_d = nc.dram_tensor("x1", [_N, _C, _NSEG, 128], bf16,
                          kind="ExternalInput")
    gt_d = nc.dram_tensor("gt", [_N, 128, _NSEG, _CA], bf16,
                          kind="ExternalInput")
    msk_d = nc.dram_tensor("msk", [128, 768], bf16, kind="ExternalInput")
    # out[v, tile-row, p(128), tile(8)*49+c]: agg accumulators + denominator
    out_d = nc.dram_tensor("out", [_N, 2, 128, 8 * _CA], bf16,
                           kind="ExternalOutput")

    with tile.TileContext(nc) as tc, ExitStack() as ctx:
        const = ctx.enter_context(tc.tile_pool(name="const", bufs=1))
        vin = ctx.enter_context(tc.tile_pool(name="vin", bufs=3))
        esb = ctx.enter_context(tc.tile_pool(name="esb", bufs=3))
        osb = ctx.enter_context(tc.tile_pool(name="osb", bufs=2))
        ps_s = ctx.enter_context(tc.tile_pool(name="ps_s", bufs=3,
                                              space="PSUM"))
        ps_o = ctx.enter_context(tc.tile_pool(name="ps_o", bufs=1,
                                              space="PSUM"))

        msk = const.tile([128, 768], bf16)

        vins = [None] * _N
        pso = [None, None]
        epair = [None] * 16

        def fetch(v, split=False):
            # all on the Sync hardware DGE queue; tw/x1 first (needed by
            # the S matmuls), gt after (only read by the later agg). View 0
            # is split per tile-row so compute starts after half the data.
            tw = vin.tile([_C, _NPIX], bf16, tag="tw", name="tw")
            x1 = vin.tile([_C, _NSEG, 128], bf16, tag="x1", name="x1")
            gt = vin.tile([128, _NSEG, _CA], bf16, tag="gt", name="gt")
            if split:
                for tr in range(2):
                    nc.sync.dma_start(tw[:, 1024 * tr:1024 * (tr + 1)],
                                      tw_d[v, :, 1024 * tr:1024 * (tr + 1)])
                    nc.sync.dma_start(x1[:, 17 * tr:17 * (tr + 1), :],
                                      x1_d[v, :, 17 * tr:17 * (tr + 1), :])
            else:
                nc.sync.dma_start(tw[:], tw_d[v])
                nc.sync.dma_start(x1[:], x1_d[v])
            nc.sync.dma_start(gt[:], gt_d[v])
            vins[v] = (tw, x1, gt)

        def s_pair(gp):
            # scores for tiles 2j, 2j+1 of view gp//8 into one 2-bank PSUM
            # tile [128q, 2*384] (tile slot stride 384); the pair shares its
            # boundary seg, so chunks (2j,k=2) and (2j+1,k=0) merge into
            # ONE matmul with a 256-wide moving operand. Then one big
            # exp + one big mask-multiply for the pair.
            v, j = gp // 8, gp % 8
            tw, x1, _ = vins[v]
            sc = ps_s.tile([128, 768], f32, tag="scat", name="sc")
            t0 = 2 * j
            tr, tc_ = t0 // 8, t0 % 8
            sb = 17 * tr + 2 * tc_          # seg of (t0, k=0)
            nc.tensor.matmul(sc[:, 0:128], lhsT=x1[:, sb, :],
                             rhs=tw[:, 128 * t0:128 * t0 + 128],
                             start=True, stop=True)
            nc.tensor.matmul(sc[:, 128:256], lhsT=x1[:, sb + 1, :],
                             rhs=tw[:, 128 * t0:128 * t0 + 128],
                             start=True, stop=True)
            nc.tensor.matmul(sc[:, 256:512], lhsT=x1[:, sb + 2, :],
                             rhs=tw[:, 128 * t0:128 * t0 + 256],
                             start=True, stop=True)
            nc.tensor.matmul(sc[:, 512:640], lhsT=x1[:, sb + 3, :],
                             rhs=tw[:, 128 * t0 + 128:128 * t0 + 256],
                             start=True, stop=True)
            nc.tensor.matmul(sc[:, 640:768], lhsT=x1[:, sb + 4, :],
                             rhs=tw[:, 128 * t0 + 128:128 * t0 + 256],
                             start=True, stop=True)
            # exp (PSUM -> SBUF bf16), then 0/1 window mask multiply
            # on DVE: exp(S)*0 == exp(S - 1e9) for out-of-window q
            e = esb.tile([128, 768], bf16, tag="e", name="e")
            epair[gp % 16] = e
            nc.scalar.activation(e[:], sc[:], AF.Exp)
            nc.vector.tensor_tensor(out=e[:], in0=e[:], in1=msk[:],
                                    op=ALU.mult)

        def a_phase(gp, slot):
            v, t = gp // 8, 2 * (gp % 8) + slot
            _, _, gt = vins[v]
            tr, tc_ = t // 8, t % 8
            if tc_ == 0:
                pso[tr] = ps_o.tile([128, 8 * _CA], f32,
                                    tag=f"pso{tr}", name=f"pso{tr}")
            po = pso[tr]
            e = epair[gp % 16]
            for k in range(3):
                seg = 17 * tr + 2 * tc_ + k
                nc.tensor.matmul(
                    po[:, _CA * tc_:_CA * (tc_ + 1)],
                    lhsT=e[:, 384 * slot + 128 * k:
                           384 * slot + 128 * (k + 1)],
                    rhs=gt[:, seg, :],
                    start=(k == 0), stop=(k == 2))
            if tc_ == 7:
                # drain the finished tile-row bank to SBUF + DRAM
                o = osb.tile([128, 8 * _CA], bf16, tag=f"ob{tr}",
                             name=f"ob{tr}")
                nc.vector.tensor_copy(o[:], po[:])
                nc.sync.dma_start(out_d[v, tr], o[:])

        # one global software pipeline over all 9*8 pairs: the PE stays 2
        # pairs ahead of the exp chain with no drain at view boundaries
        fetch(0, split=True)
        # mask DMA after view-0's (it is first needed later than tw/x1);
        # the dummy copy primes DVE's vector clock on the mask DMA: the HW
        # TensorTensor instruction has a single sync-wait slot, so the
        # first mask-mult must not need both a DMA wait and an ACT wait.
        nc.sync.dma_start(msk[:], msk_d[:])
        dummy = const.tile([128, 1], bf16)
        nc.vector.tensor_copy(dummy[:], msk[:, 0:1])
        fetch(1)
        for gp in range(74):
            if gp < 72:
                if gp % 8 == 0 and gp // 8 + 2 < _N:
                    fetch(gp // 8 + 2)
                s_pair(gp)
            if gp >= 2:
                a_phase(gp - 2, 0)
                a_phase(gp - 2, 1)
    if not nc.is_finalized():
        nc.finalize()
    return nc


def _masks() -> np.ndarray:
    """mask[q=qr*8+qc, chunk*128 + p=pr*16+pc]: 1 if q in p's 9x9 window."""
    qr = (np.arange(128) // 8)[:, None]
    qc = (np.arange(128) % 8)[:, None]
    pr = (np.arange(128) // 16)[None, :]
    pc = (np.arange(128) % 16)[None, :]
    m = np.zeros((128, 3, 128), np.float32)
    for kk in range(3):
        valid = ((qr - pr >= 0) & (qr - pr <= 8)
                 & (qc + 8 * kk - pc >= 0) & (qc + 8 * kk - pc <= 8))
        m[:, kk, :][valid] = 1.0
    m = m.reshape(128, 384)
    return np.concatenate([m, m], axis=1).astype(_BF16)  # [128, 768]


def _seg_chunk(img: np.ndarray, r0: int) -> np.ndarray:
    """img [n, ch, 136, 136] padded -> [n, ch, 34, 128] seg-major chunks
    for the 16-row strip starting at unpadded row r0."""
    n, ch = img.shape[:2]
    xs = img[:, :, r0:r0 + _SR + 8, :]                 # [n,ch,24,136]
    segs = np.empty((n, ch, _NSEG, 128), np.float32)
    for tr in range(2):
        sl = xs[:, :, 8 * tr:8 * tr + 16, :]           # [n,ch,16,136]
        sl = sl.reshape(n, ch, 16, 17, 8).transpose(0, 1, 3, 2, 4)
        segs[:, :, 17 * tr:17 * (tr + 1), :] = sl.reshape(n, ch, 17, 128)
    return segs


def kernel(**inputs) -> np.ndarray:
    A = np.asarray(inputs["A"], np.float32)            # [1,9,48,128,128]
    wc = np.asarray(inputs["warped_c"], np.float32)    # [1,9,48,128,128]
    Wt = np.asarray(inputs["Wt"], np.float32)
    Wp = np.asarray(inputs["Wp"], np.float32)
    Wg = np.asarray(inputs["Wg"], np.float32)
    Ww = np.asarray(inputs["Ww"], np.float32)

    Wtp = Wt.T @ Wp                                    # S = tw^T x1
    Wwg = Ww @ Wg
    wwgt = np.zeros((_CA, _CA), np.float32)
    wwgt[:_C, :_C] = Wwg.T
    wwgt[_C, _C] = 1.0

    # padded warped input + ones channel: [9, 49, 136, 136]
    x1p = np.pad(wc[0], ((0, 0), (0, 0), (4, 4), (4, 4)))
    x1aug = np.concatenate(
        [x1p, np.ones((_N, 1, _H + 8, _W + 8), np.float32)], axis=1)
    # g image (q-side values + ones) on host: tiny 49x49 GEMM per pixel
    gimg = np.einsum('cj,vchw->vjhw', wwgt, x1aug, optimize=True)

    # theta-folded query image on host: [9, 48, 128, 128]
    twimg = np.einsum('co,vchw->vohw', Wtp, A[0], optimize=True)

    msk = _masks()
    in_maps = []
    for cid in range(_NCORES):
        r0 = cid * _SR
        # tw tile-major: (tr, tc, pr, pc) -> [9,48,2048]
        strip = twimg[:, :, r0:r0 + _SR, :]            # [9,48,16,128]
        tw = strip.reshape(_N, _C, 2, 8, 8, 16).transpose(0, 1, 2, 4, 3, 5)
        tw = np.ascontiguousarray(tw.reshape(_N, _C, _NPIX)).astype(_BF16)
        x1segs = _seg_chunk(x1p, r0)                   # [9,48,34,128]
        gtsegs = _seg_chunk(gimg, r0)                  # [9,49,34,128]
        gt = np.ascontiguousarray(
            gtsegs.transpose(0, 3, 2, 1)).astype(_BF16)  # [9,128,34,49]
        in_maps.append({
            "tw": tw,
            "x1": np.ascontiguousarray(x1segs).astype(_BF16),
            "gt": gt,
            "msk": msk,
        })

    from concourse.bass_utils import run_bass_kernel_spmd
    if not _nc_cache:
        _nc_cache.append(_build_nc())
    res = run_bass_kernel_spmd(_nc_cache[0], in_maps, list(range(_NCORES)))
    global _last_res
    _last_res = res

    strips = []
    for cid in range(_NCORES):
        o = np.asarray(res.results[cid]["out"], np.float32)
        # o[v, tr, p=pr*16+pc, tc*49 + c]
        o = o.reshape(_N, 2, 8, 16, 8, _CA)            # v, tr, pr, pc, tc, c
        att = o[..., :_C] / o[..., _C:]
        # -> [v, c, tr, pr, tc, pc] -> [v, c, 16, 128]
        att = att.transpose(0, 5, 1, 2, 4, 3).reshape(_N, _C, _SR, _W)
        strips.append(att)
    att_full = np.concatenate(strips, axis=2)[None]    # [1,9,48,128,128]
    return (A + att_full).astype(np.float32)


# revision 29
# speedup vs baseline: 1.2336x; 1.0207x over previous
"""Trainium2 Bass kernel for DeepUnfoldingNet CTG local-window attention.

Math (per view v, per pixel p):
  theta = Wt @ A ;  phi = Wp @ x1 ;  g = Wg @ x1   (1x1 convs, C=48)
  S[p, q] = theta(p) . phi(q)  for q in the 9x9 window around p
  att = softmax_q(S);  out = Ww @ (sum_q att * g(q)) + A

Folded on HOST (the convs are tiny 48x48 GEMMs):
  tw = (Wt^T Wp)^T A        -> S = tw(p) . x1(q)
  gt = ((Ww Wg) x1)^T + ones row (softmax denominator), q-major per seg.

Sharding: H=128 -> 8 strips of 16 rows (one per core), all 9 views per core;
warped input gets a 4-pixel zero halo (matches torch-unfold zero padding).

Device tiling per view: 16 P-tiles of 8x16 pixels (=128 partitions). Each
P-tile attends over a 16x24 padded Q-window = 3 q-chunks of 128 (16 rows x 8
cols). Scores are computed transposed (S_T[q, p]) into one PSUM bank per
tile as [128q, 3*128]:
  S_T chunk k = x1[48, seg]^T-mm-tw[48, ptile]  (seg = 17*tr + 2*tc + k)
  += mask (-1e9 outside window) in ONE GpSimd tensor_tensor [128, 384]
  E = exp in ONE ScalarE activation [128, 384] (PSUM -> SBUF bf16)
  out[p, 0:49] += E_k^T-mm-gt[seg] (3-chunk PSUM accumulation, packed
  49-col regions per tile-row bank)
PE emission is software-pipelined (S of tile t+2 ahead of agg of tile t) so
the tensor engine never waits on the exp chain. Host does padding, weight
folding, layout chunking, final divide, transpose, residual.
"""

import numpy as np
import ml_dtypes

_BF16 = ml_dtypes.bfloat16

_N, _C, _H, _W = 9, 48, 128, 128
_NCORES = 8
_SR = 16            # strip rows per core
_NPIX = _SR * _W    # 2048 pixels per strip
_NSEG = 34          # 2 tile-rows x 17 col-bands of 16x8 q-chunks
_CA = _C + 1        # 48 channels + ones (denominator)

_nc_cache = []
_last_res = None


def _build_nc():
    import concourse.bacc as bacc
    import concourse.mybir as mybir
    from concourse import tile
    from contextlib import ExitStack

    f32 = mybir.dt.float32
    bf16 = mybir.dt.bfloat16
    AF = mybir.ActivationFunctionType
    ALU = mybir.AluOpType

    nc = bacc.Bacc()
    # tw: tile-major pixels (tr, tcol, pr, pc); x1: chunk-major q (seg, q)
    tw_d = nc.dram_tensor("tw", [_N, _C, _NPIX], bf16, kind="ExternalInput")
    x1_d = nc.dram_tensor("x1", [_N, _C, _NSEG, 128], bf16,
                          kind="ExternalInput")
    gt_d = nc.dram_tensor("gt", [_N, 128, _NSEG, _CA], bf16,
                          kind="ExternalInput")
    msk_d = nc.dram_tensor("msk", [128, 768], bf16, kind="ExternalInput")
    # out[v, tile-row, p(128), tile(8)*49+c]: agg accumulators + denominator
    out_d = nc.dram_tensor("out", [_N, 2, 128, 8 * _CA], bf16,
                           kind="ExternalOutput")

    with tile.TileContext(nc) as tc, ExitStack() as ctx:
        const = ctx.enter_context(tc.tile_pool(name="const", bufs=1))
        vin = ctx.enter_context(tc.tile_pool(name="vin", bufs=3))
        esb = ctx.enter_context(tc.tile_pool(name="esb", bufs=7))
        osb = ctx.enter_context(tc.tile_pool(name="osb", bufs=2))
        ps_s = ctx.enter_context(tc.tile_pool(name="ps_s", bufs=3,
                                              space="PSUM"))
        ps_o = ctx.enter_context(tc.tile_pool(name="ps_o", bufs=1,
                                              space="PSUM"))

        msk = const.tile([128, 768], bf16)

        vins = [None] * _N
        pso = [None, None]
        epair = [None] * 16

        def fetch(v, split=False):
            # all on the Sync hardware DGE queue; tw/x1 first (needed by
            # the S matmuls), gt after (only read by the later agg). View 0
            # is split per tile-row so compute starts after half the data.
            tw = vin.tile([_C, _NPIX], bf16, tag="tw", name="tw")
            x1 = vin.tile([_C, _NSEG, 128], bf16, tag="x1", name="x1")
            gt = vin.tile([128, _NSEG, _CA], bf16, tag="gt", name="gt")
            if split:
                nc.sync.dma_start(tw[:, 0:1024], tw_d[v, :, 0:1024])
                nc.sync.dma_start(x1[:, 0:9, :], x1_d[v, :, 0:9, :])
                nc.sync.dma_start(x1[:, 9:17, :], x1_d[v, :, 9:17, :])
                nc.sync.dma_start(tw[:, 1024:2048], tw_d[v, :, 1024:2048])
                nc.sync.dma_start(x1[:, 17:34, :], x1_d[v, :, 17:34, :])
            else:
                nc.sync.dma_start(tw[:], tw_d[v])
                nc.sync.dma_start(x1[:], x1_d[v])
            nc.sync.dma_start(gt[:], gt_d[v])
            vins[v] = (tw, x1, gt)

        def s_pair(gp):
            # scores for tiles 2j, 2j+1 of view gp//8 into one 2-bank PSUM
            # tile [128q, 2*384] (tile slot stride 384); the pair shares its
            # boundary seg, so chunks (2j,k=2) and (2j+1,k=0) merge into
            # ONE matmul with a 256-wide moving operand. Then one big
            # exp + one big mask-multiply for the pair.
            v, j = gp // 8, gp % 8
            tw, x1, _ = vins[v]
            sc = ps_s.tile([128, 768], f32, tag="scat", name="sc")
            t0 = 2 * j
            tr, tc_ = t0 // 8, t0 % 8
            sb = 17 * tr + 2 * tc_          # seg of (t0, k=0)
            nc.tensor.matmul(sc[:, 0:128], lhsT=x1[:, sb, :],
                             rhs=tw[:, 128 * t0:128 * t0 + 128],
                             start=True, stop=True)
            nc.tensor.matmul(sc[:, 128:256], lhsT=x1[:, sb + 1, :],
                             rhs=tw[:, 128 * t0:128 * t0 + 128],
                             start=True, stop=True)
            nc.tensor.matmul(sc[:, 256:512], lhsT=x1[:, sb + 2, :],
                             rhs=tw[:, 128 * t0:128 * t0 + 256],
                             start=True, stop=True)
            nc.tensor.matmul(sc[:, 512:640], lhsT=x1[:, sb + 3, :],
                             rhs=tw[:, 128 * t0 + 128:128 * t0 + 256],
                             start=True, stop=True)
            nc.tensor.matmul(sc[:, 640:768], lhsT=x1[:, sb + 4, :],
                             rhs=tw[:, 128 * t0 + 128:128 * t0 + 256],
                             start=True, stop=True)
            # exp (PSUM -> SBUF bf16), then 0/1 window mask multiply
            # on DVE: exp(S)*0 == exp(S - 1e9) for out-of-window q
            e = esb.tile([128, 768], bf16, tag="e", name="e")
            epair[gp % 16] = e
            nc.scalar.activation(e[:], sc[:], AF.Exp)
            nc.vector.tensor_tensor(out=e[:], in0=e[:], in1=msk[:],
                                    op=ALU.mult)

        def a_phase(gp, slot):
            v, t = gp // 8, 2 * (gp % 8) + slot
            _, _, gt = vins[v]
            tr, tc_ = t // 8, t % 8
            if tc_ == 0:
                pso[tr] = ps_o.tile([128, 8 * _CA], f32,
                                    tag=f"pso{tr}", name=f"pso{tr}")
            po = pso[tr]
            e = epair[gp % 16]
            for k in range(3):
                seg = 17 * tr + 2 * tc_ + k
                nc.tensor.matmul(
                    po[:, _CA * tc_:_CA * (tc_ + 1)],
                    lhsT=e[:, 384 * slot + 128 * k:
                           384 * slot + 128 * (k + 1)],
                    rhs=gt[:, seg, :],
                    start=(k == 0), stop=(k == 2))
            if tc_ == 7:
                # drain the finished tile-row bank to SBUF + DRAM
                o = osb.tile([128, 8 * _CA], bf16, tag=f"ob{tr}",
                             name=f"ob{tr}")
                nc.vector.tensor_copy(o[:], po[:])
                nc.sync.dma_start(out_d[v, tr], o[:])

        # one global software pipeline over all 9*8 pairs: the PE stays 2
        # pairs ahead of the exp chain with no drain at view boundaries
        fetch(0, split=True)
        # mask DMA after view-0's (it is first needed later than tw/x1);
        # the dummy copy primes DVE's vector clock on the mask DMA: the HW
        # TensorTensor instruction has a single sync-wait slot, so the
        # first mask-mult must not need both a DMA wait and an ACT wait.
        nc.sync.dma_start(msk[:], msk_d[:])
        dummy = const.tile([128, 1], bf16)
        nc.vector.tensor_copy(dummy[:], msk[:, 0:1])
        fetch(1)
        # batch 2 pairs of S per step so the PE only crosses the
        # S->agg dependency break once per two pairs
        for gp in range(0, 76, 2):
            for g in (gp, gp + 1):
                if g < 72:
                    if g % 8 == 0 and g // 8 + 2 < _N:
                        fetch(g // 8 + 2)
                    s_pair(g)
            for g in (gp - 4, gp - 3):
                if 0 <= g < 72:
                    a_phase(g, 0)
                    a_phase(g, 1)
    if not nc.is_finalized():
        nc.finalize()
    return nc


def _masks() -> np.ndarray:
    """mask[q=qr*8+qc, chunk*128 + p=pr*16+pc]: 1 if q in p's 9x9 window."""
    qr = (np.arange(128) // 8)[:, None]
    qc = (np.arange(128) % 8)[:, None]
    pr = (np.arange(128) // 16)[None, :]
    pc = (np.arange(128) % 16)[None, :]
    m = np.zeros((128, 3, 128), np.float32)
    for kk in range(3):
        valid = ((qr - pr >= 0) & (qr - pr <= 8)
                 & (qc + 8 * kk - pc >= 0) & (qc + 8 * kk - pc <= 8))
        m[:, kk, :][valid] = 1.0
    m = m.reshape(128, 384)
    return np.concatenate([m, m], axis=1).astype(_BF16)  # [128, 768]


def _seg_chunk(img: np.ndarray, r0: int) -> np.ndarray:
    """img [n, ch, 136, 136] padded -> [n, ch, 34, 128] seg-major chunks
    for the 16-row strip starting at unpadded row r0."""
    n, ch = img.shape[:2]
    xs = img[:, :, r0:r0 + _SR + 8, :]                 # [n,ch,24,136]
    segs = np.empty((n, ch, _NSEG, 128), np.float32)
    for tr in range(2):
        sl = xs[:, :, 8 * tr:8 * tr + 16, :]           # [n,ch,16,136]
        sl = sl.reshape(n, ch, 16, 17, 8).transpose(0, 1, 3, 2, 4)
        segs[:, :, 17 * tr:17 * (tr + 1), :] = sl.reshape(n, ch, 17, 128)
    return segs


def kernel(**inputs) -> np.ndarray:
    A = np.asarray(inputs["A"], np.float32)            # [1,9,48,128,128]
    wc = np.asarray(inputs["warped_c"], np.float32)    # [1,9,48,128,128]
    Wt = np.asarray(inputs["Wt"], np.float32)
    Wp = np.asarray(inputs["Wp"], np.float32)
    Wg = np.asarray(inputs["Wg"], np.float32)
    Ww = np.asarray(inputs["Ww"], np.float32)

    Wtp = Wt.T @ Wp                                    # S = tw^T x1
    Wwg = Ww @ Wg
    wwgt = np.zeros((_CA, _CA), np.float32)
    wwgt[:_C, :_C] = Wwg.T
    wwgt[_C, _C] = 1.0

    # padded warped input + ones channel: [9, 49, 136, 136]
    x1p = np.pad(wc[0], ((0, 0), (0, 0), (4, 4), (4, 4)))
    x1aug = np.concatenate(
        [x1p, np.ones((_N, 1, _H + 8, _W + 8), np.float32)], axis=1)
    # g image (q-side values + ones) on host: tiny 49x49 GEMM per pixel
    gimg = np.einsum('cj,vchw->vjhw', wwgt, x1aug, optimize=True)

    # theta-folded query image on host: [9, 48, 128, 128]
    twimg = np.einsum('co,vchw->vohw', Wtp, A[0], optimize=True)

    msk = _masks()
    in_maps = []
    for cid in range(_NCORES):
        r0 = cid * _SR
        # tw tile-major: (tr, tc, pr, pc) -> [9,48,2048]
        strip = twimg[:, :, r0:r0 + _SR, :]            # [9,48,16,128]
        tw = strip.reshape(_N, _C, 2, 8, 8, 16).transpose(0, 1, 2, 4, 3, 5)
        tw = np.ascontiguousarray(tw.reshape(_N, _C, _NPIX)).astype(_BF16)
        x1segs = _seg_chunk(x1p, r0)                   # [9,48,34,128]
        gtsegs = _seg_chunk(gimg, r0)                  # [9,49,34,128]
        gt = np.ascontiguousarray(
            gtsegs.transpose(0, 3, 2, 1)).astype(_BF16)  # [9,128,34,49]
        in_maps.append({
            "tw": tw,
            "x1": np.ascontiguousarray(x1segs).astype(_BF16),
            "gt": gt,
            "msk": msk,
        })

    from concourse.bass_utils import run_bass_kernel_spmd
    if not _nc_cache:
        _nc_cache.append(_build_nc())
    res = run_bass_kernel_spmd(_nc_cache[0], in_maps, list(range(_NCORES)))
    global _last_res
    _last_res = res

    strips = []
    for cid in range(_NCORES):
        o = np.asarray(res.results[cid]["out"], np.float32)
        # o[v, tr, p=pr*16+pc, tc*49 + c]
        o = o.reshape(_N, 2, 8, 16, 8, _CA)            # v, tr, pr, pc, tc, c
        att = o[..., :_C] / o[..., _C:]
        # -> [v, c, tr, pr, tc, pc] -> [v, c, 16, 128]
        att = att.transpose(0, 5, 1, 2, 4, 3).reshape(_N, _C, _SR, _W)
        strips.append(att)
    att_full = np.concatenate(strips, axis=2)[None]    # [1,9,48,128,128]
    return (A + att_full).astype(np.float32)
